# revision 20
# baseline (speedup 1.0000x reference)
"""Trainium2 Bass kernel for nn_CNN_V1_32796370272431.

Math (see reference):
    h   = relu(const_vec @ W1^T + b1)          # [F, HID]       tiny
    k1  = einsum('fh,fsh->fs', h, W2) + b2     # [F, S]         tiny
    k2  = k1 @ smooth                          # [F, S]         tiny
    outs= einsum('bsf,fs->bf', x, k2)          # [B, F]         268MB of x -> memory bound
    out = relu(outs @ fcW1.T + fcb1) @ fcW2.T + fcb2   # [B, 1] tiny

k2 and the fc weights are folded on the host; the device streams x once
and contracts it against k2.

Device structure ("diagonal matmul"): split s into 32 chunks of 128.
For chunk c the PE stationary is K_c[s, j] = k2[j, c*128+s] ([128, 64])
and the moving operand is x for that chunk, laid out [128 s, NP*64
(b-major, f-minor)] so ONE weight load serves every batch row.  The MM
computes psum[j, b*64+f] += sum_s k2[j, s] x[b, s, f]; accumulating all
chunks, the diagonal j==f is exactly outs[b, f].  The off-diagonal
entries are discarded by a mask multiply + segmented reduce on DVE
(psum -> ACT fp16 drain -> DVE mask-mul 2x -> DVE 64-run reduce ->
outsT [64 f, b]).  The fc head runs on the transposed outs directly
(biases become per-partition ACT biases).

Because PE does multiply+reduce in the matmul itself, there is no bulk
DVE elementwise pass, and the moving operand can be fp8: x streams as
FP8_E3M4 (4 mantissa bits), quantized on the host with first-order
error feedback along s.  k2 is smoothed by a 5-tap low-pass along s, so
noise-shaping the quantization error to high frequency cancels most of
it in the dot product (measured end-to-end rel err 7.5e-3 vs 2e-2
tolerance; the stationary stays fp16, PE upconverts both to ~fp22).
fp8 halves HBM bytes; PE at 1 col/cycle needs ~27.5us for the 8.4MB
shard, so NV=4 batch rows are offloaded to DVE (tensor_tensor against a
replicated k2 tile + 64-run segmented reduce, partition-summed by tiny
one-hot PE matmuls), leaving PE ~23.9us / DVE ~19us under the measured
DMA wall of ~26us (the x stream sustains ~320-350 GB/s/core; HBM-bound).

Pipeline notes (measured on this part):
- The pass tail (fc head + output DMA) of pass N is emitted into pass
  N+1's instruction stream after the first DMA group's matmuls: engine
  queues are strict FIFO, so an inline tail would stall next-pass chunk
  matmuls on the DVE diagonal-extract chain (~3us/pass).
- drain="dve": the diagonal mask-mul reads PSUM directly; the ACT-copy
  drain holds PSUM ~1us longer and serializes next-pass matmul c0.
- The tiny [1,32] output DMA carries a ~2us HBM write-receipt latency;
  anything queued behind it on the same engine stalls.  Bench reps
  rotate output slots; the real kernel writes once at the end.
- xv (the DVE rows) must load at the head of the pass: DVE chews it for
  ~19us and the PSUM-freeing mask-mul queues behind those ops.

Each core handles 32 batch rows; output is [1, 32] per core.
"""

import numpy as np

import concourse.bass as bass
import concourse.mybir as mybir
from concourse.bass_utils import run_bass_kernel_spmd
from concourse.tile import TileContext

# Problem constants (hardcoded per harness contract).
B, S, F, HID = 256, 4096, 64, 10
N_CORES = 8
NB = B // N_CORES            # batch rows per core = 32
NCHUNK = S // 128            # s-chunks of 128 = 32
GC = 2                       # chunks per DMA group (default)
XBUFS = 17                   # x-tile pool depth (a full pass in flight)
VCOLS = F * 32               # free cols of a DVE-offload row tile = 2048

F32 = mybir.dt.float32
F16 = mybir.dt.float16
E3 = mybir.dt.float8e3

_PROGRAM_CACHE = {}
# Stream dtype for x ("f16" or "e3") and number of batch rows offloaded
# from PE to DVE (only useful for the fp8 stream, where PE is the
# bottleneck engine otherwise).
STREAM = "e3"
NV = 4
# Max matmul moving width (hardware allows 1024 for 8/16-bit moving).
MMW = 512
# Alternate PE tile_position between chunks: odd chunks write PSUM
# partitions 64-127, letting the weight load of one half overlap the
# other half's matmul streaming (manual weight double-buffering; walrus
# runs with ldw-opt disabled).  The fc-head contraction sums the halves
# for free via a row-duplicated fcW1.
ALT_TILE = True
# DMA ring topology: "sa" = sync+ACT HWDGE rings for x, xv on sync;
# "sg" = sync+Pool(SWDGE) rings for x, xv on the ACT ring.  Each HWDGE
# ring tops out around ~205 GB/s, so three streams beat two.
RINGS = "sa"
# Who drains PSUM for the diagonal extraction: "dve" = mask-mul reads
# PSUM directly (frees PSUM ~1us sooner than the ACT-copy path and skips
# the ACT round trip); "act" = ACT copies PSUM->SBUF fp16 first.
DRAIN = "dve"
# Stream the DVE-offload rows as fp16 instead of fp8: their
# tensor_tensor runs in 2x mode (fp8 sources are capped at 1x).
XV16 = False
# Issue the fp8 x DMAs under a bitcast fp16 view (same bytes): 1-byte
# dtypes measurably degrade the DMA path on this part.
DMA16 = True
# Engine that issues the final [1, NB] output DMA.  (Pool/SWDGE cannot
# carry loop sync attachments on this walrus build: "ISA wrong length".)
OUT_ENG = "sync"
# Emit pass N's fc head + output into pass N+1's instruction stream, after
# the first DMA group's matmuls.  PE is strict FIFO for matmuls, so an
# inline fc head makes next-pass chunk matmuls wait on the DVE diagonal
# extract chain (~3us) every pass.
TAIL_DEFER = True
# PSUM accumulator double buffering (2 = manual mega-tile).  Measured
# neutral-to-worse in the full pipeline; keep 1 with drain="dve".
ABUFS = 1
# Split the xv transfer across both HWDGE rings to balance ring bytes.
XV_SPLIT = False
# Emit the xv DMA after the x group loop (bad: DVE needs it early).
XV_LATE = False
# Interleave per-row xv DMAs between the first nv x groups: spreads the
# burst and lets each pv-mul start as soon as its row lands.
XV_INTER = True


def _dma(eng, out, in_, dma16):
    if dma16 == 32 and out.dtype == E3:
        eng.dma_start(out=out.bitcast(F32), in_=in_.bitcast(F32))
    elif dma16 and out.dtype == E3:
        eng.dma_start(out=out.bitcast(F16), in_=in_.bitcast(F16))
    else:
        eng.dma_start(out=out, in_=in_)


def _np_e3():
    import ml_dtypes

    return ml_dtypes.float8_e3m4


def _stream_np(stream):
    return _np_e3() if stream == "e3" else np.float16


def _stream_dt(stream):
    return E3 if stream == "e3" else F16


def _split_excess_waits(nc):
    """Walrus (this build) accepts at most one sync-wait per instruction
    (two on InstEventSemaphore), but the Tile scheduler can attach more.
    Move the excess onto same-engine InstNoOps placed immediately before
    the instruction — identical semantics, since the engine sequencer
    executes its stream in order."""
    for fn in nc.m.functions:
        for bb in fn.blocks:
            out = []
            changed = False
            for ins in bb.instructions:
                si = ins.sync_info
                if isinstance(ins, mybir.InstISA) or (
                    ins.engine == mybir.EngineType.Pool
                    and isinstance(ins, mybir.InstDMACopy)
                ):
                    # walrus lowers Pool DMAs (SWDGE) and InstISA to raw
                    # ISA code and rejects sync-waits on them ("ISA wrong
                    # length") — carry the waits on a Pool NoOp instead.
                    cap = 0
                elif isinstance(ins, mybir.InstEventSemaphore):
                    cap = 2
                else:
                    cap = 1
                if si is not None and si.on_wait and len(si.on_wait) > cap:
                    waits = list(si.on_wait)
                    keep = waits[-cap:] if cap else []
                    for w in (waits[:-cap] if cap else waits):
                        nop = mybir.InstNoOp(
                            name=nc.get_next_instruction_name(),
                            engine=ins.engine,
                            bass_nofuse=True,
                            sync_info=mybir.SyncInfo(on_wait=[w], on_update=[]),
                        )
                        nc.register_instruction(nop, overwrite=True)
                        out.append(nop)
                    si.on_wait = keep
                    changed = True
                out.append(ins)
            if changed:
                bb.instructions = out


def _build_program(
    reps=1, loop_iters=0, stream=STREAM, nv=NV, xbufs=XBUFS, dual_ring=True,
    mmw=MMW, alt_tile=ALT_TILE, gc=GC, rings=RINGS, drain=DRAIN, xv16=XV16,
    dma16=DMA16, skip_compute=False, skip_dma=False, dma_groups=0,
    out_mode="all", out_eng=OUT_ENG, tail_defer=TAIL_DEFER, abufs=ABUFS,
    xv_split=XV_SPLIT, xv_late=XV_LATE, xv_inter=XV_INTER,
    skip_dve=False, skip_pe=False,
):
    """Build the (SPMD, per-core) bass program once; inputs are DRAM params.

    reps > 1 repeats the full streaming loop (for benchmarking: the
    marginal wall time per extra rep is the steady-state kernel time,
    free of dispatch/transfer overhead).  loop_iters > 0 additionally
    wraps the reps bodies in a hardware For_i loop."""
    nc = bass.Bass(trn_type="TRN2", target_bir_lowering=False)

    x_dt = _stream_dt(stream)
    xv_dt = F16 if xv16 else x_dt
    np_ = NB - nv                 # batch rows handled by PE
    pecols = np_ * F              # moving cols per chunk

    acc_p = 128 if alt_tile else F  # psum partitions used by the chunk MMs

    ngrp = NCHUNK // gc
    x_d = nc.declare_dram_parameter(
        "x", [ngrp, 128, gc * pecols], x_dt, isOutput=False)
    if nv:
        xv_d = nc.declare_dram_parameter(
            "xv", [128, nv * VCOLS], xv_dt, isOutput=False)
    k_d = nc.declare_dram_parameter("kst", [128, NCHUNK * F], F16, isOutput=False)
    mk_d = nc.declare_dram_parameter("mask", [acc_p, pecols], F16, isOutput=False)
    if nv:
        mv_d = nc.declare_dram_parameter("mv", [128, VCOLS], F16, isOutput=False)
        on_d = nc.declare_dram_parameter("ones", [128, 1], F32, isOutput=False)
    w1_d = nc.declare_dram_parameter("fcW1d", [acc_p, HID], F32, isOutput=False)
    b1_d = nc.declare_dram_parameter("fcb1b", [HID, NB], F32, isOutput=False)
    w2_d = nc.declare_dram_parameter("fcW2T", [HID, 1], F32, isOutput=False)
    b2_d = nc.declare_dram_parameter("fcb2b", [1, NB], F32, isOutput=False)
    # Bench programs (reps>1) rotate the output slot per rep so the tiny
    # out DMA's ~2us HBM write-completion never chains pass-to-pass (the
    # real reps=1 kernel writes out once; no WAW exists there).
    if reps > 1:
        out_d = nc.declare_dram_parameter("out", [reps, 1, NB], F32,
                                          isOutput=True)
    else:
        out_d = nc.declare_dram_parameter("out", [1, NB], F32, isOutput=True)

    with TileContext(nc) as tc:
        with (
            tc.tile_pool(name="const", bufs=1) as cpool,
            tc.tile_pool(name="xin", bufs=xbufs) as xpool,
            tc.tile_pool(name="xvin", bufs=2) as vpool,
            tc.tile_pool(name="small", bufs=1) as spool,
            tc.tile_pool(name="tail", bufs=2) as tpool,
            tc.tile_pool(name="acc", bufs=1, space="PSUM") as apool,
            tc.tile_pool(name="ptail", bufs=1, space="PSUM") as ppool,
        ):
            k_sb = cpool.tile([128, NCHUNK * F], F16)
            mk_sb = cpool.tile([acc_p, pecols], F16)
            w1_sb = cpool.tile([acc_p, HID], F32)
            b1_sb = cpool.tile([HID, NB], F32)
            w2_sb = cpool.tile([HID, 1], F32)
            b2_sb = cpool.tile([1, NB], F32)
            # Const loads on the ACT HWDGE ring so they overlap with the
            # x stream on the SP ring from the very first instruction.
            nc.scalar.dma_start(out=k_sb[:], in_=k_d[:])
            nc.scalar.dma_start(out=mk_sb[:], in_=mk_d[:])
            nc.scalar.dma_start(out=w1_sb[:], in_=w1_d[:])
            nc.scalar.dma_start(out=b1_sb[:], in_=b1_d[:])
            nc.scalar.dma_start(out=w2_sb[:], in_=w2_d[:])
            nc.scalar.dma_start(out=b2_sb[:], in_=b2_d[:])
            if nv:
                mv_sb = cpool.tile([128, VCOLS], F16)
                on_sb = cpool.tile([128, 1], F32)
                nc.scalar.dma_start(out=mv_sb[:], in_=mv_d[:])
                nc.scalar.dma_start(out=on_sb[:], in_=on_d[:])
            else:
                mv_sb = on_sb = None

            # outsT lives across passes; rows [64:128, np_:NB] are never
            # written when alt_tile (zeroed once here).
            outsT = cpool.tile([acc_p, NB], F32)
            nc.vector.memset(outsT[:], 0.0)

            # One PSUM bank holds all the small matmul outputs:
            # cols [0:nv) acc2 one-hots, [nv:nv+NB) fc1, [nv+NB:nv+2NB) fc2.
            psml = ppool.tile([F, nv + 2 * NB], F32)
            # abufs=2: manual PSUM double buffer.  One [acc_p, 2*pecols]
            # tile (7 banks at pecols=1792); passes alternate column
            # halves, so next pass's matmuls never wait on this pass's
            # PSUM drain.  256-col matmul slices keep every write inside
            # one bank (the odd half is not 512-aligned).
            psmega = (
                ppool.tile([acc_p, 2 * pecols], F32, name="mega")
                if abufs == 2 else None
            )

            # Compute-only probe: x resident in SBUF, loaded once.
            xt_static = xv_static = None
            if skip_dma:
                xt_static = cpool.tile([128, gc * pecols], x_dt)
                nc.sync.dma_start(out=xt_static[:], in_=x_d[0])
                if nv:
                    xv_static = cpool.tile([128, nv * VCOLS], xv_dt)
                    nc.sync.dma_start(out=xv_static[:], in_=xv_d[:])

            args = (nc, x_d, xv_d if nv else None, out_d, k_sb, mk_sb, mv_sb,
                    on_sb, w1_sb, b1_sb, w2_sb, b2_sb, apool, psml, psmega,
                    outsT, xpool, vpool, spool, tpool, ppool, xt_static,
                    xv_static)

            def _bodies():
                pending = [None]
                for _rep in range(reps):
                    write_out = out_mode == "all" or _rep == reps - 1
                    pending[0] = _pass_body(
                        *args, stream=stream, nv=nv,
                        dual_ring=dual_ring, mmw=mmw, alt_tile=alt_tile,
                        gc=gc, rings=rings, drain=drain, xv16=xv16,
                        dma16=dma16, skip_compute=skip_compute,
                        skip_dma=skip_dma, dma_groups=dma_groups,
                        write_out=write_out, tail_defer=tail_defer,
                        pending_tail=pending[0],
                        out_slot=(_rep if reps > 1 else None),
                        xv_split=xv_split, xv_late=xv_late,
                        xv_inter=xv_inter, acc_half=_rep % 2,
                        skip_dve=skip_dve, skip_pe=skip_pe)
                # reps done: if the loop wraps (For_i), the deferred tail of
                # the last rep runs at the START of the next iteration (same
                # program point; SBUF state persists).  Without a loop, flush.
                return pending[0]

            hints = (
                mybir.EngineType.PE,
                mybir.EngineType.DVE,
                mybir.EngineType.SP,
                mybir.EngineType.Activation,
            ) + (
                (mybir.EngineType.Pool,)
                if (rings == "sg" or out_eng == "gpsimd") else ()
            )
            if loop_iters:
                with tc.For_i(0, loop_iters, 1, hint_engines=hints):
                    # The last rep's deferred tail is dropped: flushing it at
                    # the end of the body would reintroduce the inline stall,
                    # and out_d was already written by the previous rep with
                    # identical data (the bench repeats the same pass).
                    _bodies()
            else:
                tail = _bodies()
                if tail is not None:
                    tail(None)

    _split_excess_waits(nc)
    return nc


def _pass_body(nc, x_d, xv_d, out_d, k_sb, mk_sb, mv_sb, on_sb,
               w1_sb, b1_sb, w2_sb, b2_sb, apool, psml, psmega, outsT,
               xpool, vpool, spool, tpool, ppool, xt_static, xv_static,
               stream, nv,
               dual_ring, mmw, alt_tile, gc, rings, drain, xv16, dma16,
               skip_compute, skip_dma, dma_groups=0, write_out=True,
               tail_defer=True, pending_tail=None, out_slot=None,
               xv_split=False, xv_late=False, xv_inter=False, acc_half=0,
               skip_dve=False, skip_pe=False):
    """Emit one full streaming pass.  Returns the pass's fc-head closure if
    it was deferred (to be emitted after the NEXT pass's first DMA group's
    matmuls), else None.  pending_tail is the previous pass's closure."""
    x_dt = _stream_dt(stream)
    xv_dt = F16 if xv16 else x_dt
    np_ = NB - nv
    pecols = np_ * F

    # x DMAs ride two compute-free descriptor streams; the ACT drain rides
    # the ACT engine.  No x-load ever queues behind a compute dependency
    # from the previous pass.
    acc_p = 128 if alt_tile else F
    dr_sb = None
    if drain in ("act", "act2") and not skip_compute:
        dr_sb = spool.tile([acc_p, pecols], F16, name="dr")
    prod_sb = None
    if drain == "dve2" and not skip_compute and not skip_pe:
        prod_sb = spool.tile([acc_p, pecols], F16, name="prod2")

    def _emit_xv_row(xv_t, v):
        _dma(nc.sync, xv_t[:, v * VCOLS : (v + 1) * VCOLS],
             xv_d[:, v * VCOLS : (v + 1) * VCOLS], dma16)

    def _emit_xv(xv_t):
        if xv_split:
            half = nv * VCOLS // 2
            _dma(nc.sync, xv_t[:, 0:half], xv_d[:, 0:half], dma16)
            _dma(nc.scalar, xv_t[:, half:], xv_d[:, half:], dma16)
        else:
            xv_eng = nc.scalar if rings == "sg" else nc.sync
            _dma(xv_eng, xv_t[:], xv_d[:], dma16)

    xv_t = None
    if nv and not skip_dma:
        # (also emitted in skip_compute probes so DMA-only measures the
        # full byte stream)
        xv_t = vpool.tile([128, nv * VCOLS], xv_dt)
        if not xv_late and not xv_inter:
            _emit_xv(xv_t)
    elif nv and skip_dma:
        xv_t = xv_static

    acc = None
    if not skip_compute:
        if psmega is not None:
            acc = psmega[:, acc_half * pecols : (acc_half + 1) * pecols]
            mmw = min(mmw, 256)
        else:
            acc = apool.tile([acc_p, pecols], F32)

    alt_eng = nc.gpsimd if rings == "sg" else nc.scalar
    ngrp_dma = dma_groups if (skip_compute and dma_groups) else NCHUNK // gc
    for g in range(ngrp_dma):
        if skip_dma:
            xt = xt_static
        else:
            xt = xpool.tile([128, gc * pecols], x_dt)
            dma_eng = alt_eng if (dual_ring and g % 2) else nc.sync
            _dma(dma_eng, xt[:], x_d[g], dma16)
        if skip_compute:
            continue
        if skip_pe:
            if g == 0 and pending_tail is not None:
                pending_tail(None)
                pending_tail = None
            continue
        for cc in range(gc):
            c = g * gc + cc
            if alt_tile and c % 2:
                prow, tpos = F, (0, 64)
            else:
                prow, tpos = 0, (0, 0)
            start = c < 2 if alt_tile else c == 0
            stop = c >= NCHUNK - 2 if alt_tile else c == NCHUNK - 1
            q0 = 0
            while q0 < pecols:
                q1 = min(q0 + mmw, pecols)
                nc.tensor.matmul(
                    out=acc[prow : prow + F, q0:q1],
                    lhsT=k_sb[:, c * F : (c + 1) * F],
                    rhs=xt[:, cc * pecols + q0 : cc * pecols + q1],
                    start=start,
                    stop=stop,
                    tile_position=tpos if alt_tile else None,
                )
                q0 = q1
            if (drain == "act2" and alt_tile and c == NCHUNK - 2
                    and not skip_compute):
                nc.scalar.copy(out=dr_sb[0:F, :], in_=acc[0:F, :])
            if (drain == "dve2" and alt_tile and c == NCHUNK - 2
                    and not skip_compute):
                nc.vector.tensor_mul(
                    out=prod_sb[0:F, :], in0=acc[0:F, :], in1=mk_sb[0:F, :])
        if xv_inter and nv and not skip_dma and g < nv:
            _emit_xv_row(xv_t, g)
        if g == 0 and pending_tail is not None:
            # Previous pass's fc head: its inputs (outsT) become ready
            # while this group's matmuls stream, so PE never stalls.
            pending_tail(None)
            pending_tail = None

    if xv_late and nv and not skip_dma:
        _emit_xv(xv_t)

    if skip_compute:
        if pending_tail is not None:
            pending_tail(None)
        # DMA-only probe: produce a dummy output from consts.
        if write_out:
            out_sb = spool.tile([1, NB], F32)
            nc.vector.tensor_copy(out=out_sb[:], in_=k_sb[0:1, 0:NB])
            dst = out_d[:] if out_slot is None else out_d[out_slot]
            nc.sync.dma_start(out=dst, in_=out_sb[:])
        return None
    if pending_tail is not None:
        pending_tail(None)
        pending_tail = None

    # DVE offload rows: products + segmented reduce, partition-summed by
    # tiny one-hot matmuls.  Runs on DVE concurrently with the chunk MMs.
    rv_sbs = []
    for v in range(nv if not skip_dve else 0):
        pv = spool.tile([128, VCOLS], F16, name=f"pv{v}")
        nc.vector.tensor_mul(
            out=pv[:], in0=xv_t[:, v * VCOLS : (v + 1) * VCOLS], in1=mv_sb[:])
        rv = spool.tile([128, F], F32, name=f"rv{v}")
        nc.vector.tensor_reduce(
            out=rv[:],
            in_=pv[:].rearrange("p (f t) -> p f t", f=F),
            axis=mybir.AxisListType.X,
            op=mybir.AluOpType.add,
        )
        rv_sbs.append(rv)
    for v, rv in enumerate(rv_sbs):
        nc.tensor.matmul(
            out=psml[:, v : v + 1], lhsT=rv[:], rhs=on_sb[:],
            start=True, stop=True,
        )

    # Diagonal extraction: mask-mul + 64-run segmented reduce into
    # outsT[f, b].  When alt_tile, outsT has 128 rows (two psum halves);
    # the fc1 contraction sums them via the row-duplicated fcW1.
    if skip_pe:
        pass
    elif drain == "act2" and alt_tile:
        # Low half was copied right after chunk NCHUNK-2 (in the group
        # loop); here only the odd half remains.
        nc.scalar.copy(out=dr_sb[F:, :], in_=acc[F:, :])
        src = dr_sb
    elif drain in ("act", "act2"):
        nc.scalar.copy(out=dr_sb[:], in_=acc[:])
        src = dr_sb
    else:
        src = acc
    if not skip_pe:
        if drain == "dve2" and alt_tile:
            # Low half already multiplied (after chunk NCHUNK-2); finish
            # the odd half, then reduce the whole product tile.
            prod = prod_sb
            nc.vector.tensor_mul(
                out=prod[F:, :], in0=acc[F:, :], in1=mk_sb[F:, :])
        else:
            prod = spool.tile([acc_p, pecols], F16)
            nc.vector.tensor_mul(out=prod[:], in0=src[:], in1=mk_sb[:])
        nc.vector.tensor_reduce(
            out=outsT[:, 0:np_],
            in_=prod[:].rearrange("j (i f) -> j i f", f=F),
            axis=mybir.AxisListType.X,
            op=mybir.AluOpType.add,
        )
    if nv and not skip_dve:
        nc.vector.tensor_copy(out=outsT[0:F, np_:NB], in_=psml[:, 0:nv])

    if not write_out:
        return None

    def _tail(_):
        # fc head on outsT [acc_p, NB]; biases are host-broadcast tiles
        # added on DVE (ACT stays instruction-free inside the loop).
        hh_ps = psml[0:HID, nv : nv + NB]
        nc.tensor.matmul(
            out=hh_ps, lhsT=w1_sb[:], rhs=outsT[:], start=True, stop=True
        )
        hhb = tpool.tile([HID, NB], F32, name="hhb")
        nc.vector.tensor_add(out=hhb[:], in0=hh_ps, in1=b1_sb[:])
        hhT_sb = tpool.tile([HID, NB], F32, name="hht")
        nc.vector.tensor_relu(out=hhT_sb[:], in_=hhb[:])
        f_ps = psml[0:1, nv + NB : nv + 2 * NB]
        nc.tensor.matmul(
            out=f_ps, lhsT=w2_sb[:], rhs=hhT_sb[:], start=True, stop=True
        )
        out_sb = tpool.tile([1, NB], F32, name="osb")
        nc.vector.tensor_add(out=out_sb[:], in0=f_ps, in1=b2_sb[:])
        dst = out_d[:] if out_slot is None else out_d[out_slot]
        nc.sync.dma_start(out=dst, in_=out_sb[:])

    if tail_defer:
        return _tail
    _tail(None)
    return None


def _fold_k2(W1, b1, W2, b2, const_vec, smooth):
    h = np.maximum(np.einsum("c,fhc->fh", const_vec, W1) + b1, 0.0)
    k1 = np.einsum("fh,fsh->fs", h.astype(np.float32), W2) + b2
    return (k1.astype(np.float32) @ smooth).astype(np.float32)  # [F, S]


def _host_consts(k2, fcW1, fcb1, fcW2, fcb2, nv, alt_tile=ALT_TILE):
    """Device-side constant tensors shared by all cores."""
    np_ = NB - nv
    pecols = np_ * F
    acc_p = 128 if alt_tile else F

    # PE stationaries: kst[p, c*F + j] = k2[j, c*128 + p]
    kst = np.ascontiguousarray(
        k2.reshape(F, NCHUNK, 128).transpose(2, 1, 0).reshape(128, NCHUNK * F),
        dtype=np.float16)

    # Diagonal mask over (i-batch-row, f) blocks; with alt_tile the same
    # diagonal repeats on psum partitions 64-127.
    mask = np.zeros((acc_p, pecols), np.float16)
    for j in range(F):
        mask[j, j::F] = 1.0
        if alt_tile:
            mask[F + j, j::F] = 1.0

    w1 = np.ascontiguousarray(fcW1.T, dtype=np.float32)     # [F, HID]
    w1d = np.concatenate([w1, w1], axis=0) if alt_tile else w1

    consts = {
        "kst": kst,
        "mask": mask,
        "fcW1d": np.ascontiguousarray(w1d),
        "fcb1b": np.ascontiguousarray(
            np.broadcast_to(fcb1.reshape(HID, 1), (HID, NB)), dtype=np.float32),
        "fcW2T": np.ascontiguousarray(fcW2.T, dtype=np.float32),
        "fcb2b": np.ascontiguousarray(
            np.broadcast_to(np.reshape(fcb2, (1, 1)), (1, NB)), dtype=np.float32),
    }
    if nv:
        # DVE-offload k2 tile: mv[p, f*32 + t] = k2[f, p*32 + t]
        consts["mv"] = np.ascontiguousarray(
            k2.reshape(F, 128, 32).transpose(1, 0, 2).reshape(128, VCOLS),
            dtype=np.float16)
        consts["ones"] = np.ones((128, 1), np.float32)
    return consts


def _quantize_x(x, stream):
    """Host-side conversion of x to the stream dtype.  For fp8 the
    quantization runs with first-order error feedback along s: the
    carried residual noise-shapes the error to high frequencies, where
    the 5-tap-smoothed k2 barely responds."""
    xf = np.asarray(x, dtype=np.float32)
    if stream == "f16":
        return xf.astype(np.float16)
    qdt = _np_e3()
    q = np.empty(xf.shape, qdt)
    e = np.zeros((xf.shape[0], xf.shape[2]), np.float32)
    for s in range(xf.shape[1]):
        v = xf[:, s, :] + e
        qs = v.astype(qdt)
        q[:, s, :] = qs
        e = v - qs.astype(np.float32)
    return q


def _layout_core(xq_core, nv, gc=GC, xv_core=None):
    """Per-core device layouts from the core's [NB, S, F] quantized shard.

    PE rows (first NB-nv): [ngrp, 128, gc*pecols] with free index
    cc*pecols + b*F + f and partition p = s within chunk.
    DVE rows (last nv): [128, nv*VCOLS] with partition p = s//32 and
    free v*VCOLS + f*32 + (s%32)."""
    np_ = NB - nv
    ngrp = NCHUNK // gc
    pe = xq_core[:np_]                         # [np, S, F]
    arr = pe.reshape(np_, ngrp, gc, 128, F)    # b g cc p f
    arr = arr.transpose(1, 3, 2, 0, 4)         # g p cc b f
    x_pe = np.ascontiguousarray(arr.reshape(ngrp, 128, gc * np_ * F))
    maps = {"x": x_pe}
    if nv:
        dv = xq_core[np_:] if xv_core is None else xv_core  # [nv, S, F]
        arr = dv.reshape(nv, 128, 32, F)       # v p t f
        arr = arr.transpose(1, 0, 3, 2)        # p v f t
        maps["xv"] = np.ascontiguousarray(arr.reshape(128, nv * VCOLS))
    return maps


def make_in_maps(inputs, stream=STREAM, nv=NV, alt_tile=ALT_TILE, gc=GC,
                 xv16=XV16):
    """Full input dict -> per-core in_maps for run_bass_kernel_spmd."""
    k2 = _fold_k2(
        *(np.asarray(inputs[k], dtype=np.float32)
          for k in ("W1", "b1", "W2", "b2", "const_vec", "smooth")))
    consts = _host_consts(
        k2,
        *(np.asarray(inputs[k], dtype=np.float32)
          for k in ("fcW1", "fcb1", "fcW2", "fcb2")),
        nv=nv, alt_tile=alt_tile)
    x_full = np.asarray(inputs["x"])
    xq = _quantize_x(x_full, stream)
    xf16 = x_full.astype(np.float16) if (xv16 and nv) else None
    in_maps = []
    for c in range(N_CORES):
        xv_core = None
        if xf16 is not None:
            xv_core = xf16[c * NB + NB - nv : (c + 1) * NB]
        maps = _layout_core(xq[c * NB : (c + 1) * NB], nv, gc, xv_core)
        in_maps.append({**maps, **consts})
    return in_maps


def _enable_jit_cache():
    try:
        import os
        import jax

        cache = os.environ.get("BASS_JIT_CACHE_DIR", "/tmp/jax_bass_cache")
        jax.config.update("jax_compilation_cache_dir", cache)
        jax.config.update("jax_persistent_cache_min_entry_size_bytes", -1)
        jax.config.update("jax_persistent_cache_min_compile_time_secs", 0.5)
    except Exception:
        pass


def run(inputs, trace=False, reps=1, stream=STREAM, nv=NV, alt_tile=ALT_TILE,
        mmw=MMW, gc=GC, rings=RINGS, drain=DRAIN, xv16=XV16, dma16=DMA16,
        **run_kwargs):
    """Run on 8 NeuronCores; returns (full_output, BassKernelResults)."""
    _enable_jit_cache()
    key = ("prog", reps, stream, nv, alt_tile, mmw, gc, rings, drain, xv16,
           dma16)
    if key not in _PROGRAM_CACHE:
        _PROGRAM_CACHE[key] = _build_program(
            reps=reps, stream=stream, nv=nv, alt_tile=alt_tile, mmw=mmw, gc=gc,
            rings=rings, drain=drain, xv16=xv16, dma16=dma16)
    nc = _PROGRAM_CACHE[key]

    in_maps = make_in_maps(inputs, stream=stream, nv=nv, alt_tile=alt_tile,
                           gc=gc, xv16=xv16)
    core_ids = list(range(N_CORES))
    res = run_bass_kernel_spmd(nc, in_maps, core_ids, trace=trace, **run_kwargs)
    out = np.concatenate(
        [np.asarray(res.results[c]["out"]).reshape(NB) for c in core_ids]
    )
    return out.reshape(B, 1).astype(np.float32), res


def kernel(**inputs) -> np.ndarray:
    out, _ = run(inputs)
    return out

